# revision 1
# baseline (speedup 1.0000x reference)
"""Bass/Trainium2 kernel for nn_BiasedAxialAttention (triangle attention, is_row).

Self-contained: builds a Bass/Tile SPMD program, shards inputs over 8
NeuronCores host-side, runs via run_bass_kernel_spmd, gathers the output.

Sharding: the tied contraction axis n (pair columns) is split 8 ways.
Each core computes a partial [H, L, L] logit tensor, the partials are
AllReduced (host-precomputed bias@Wb is folded into the reduction), and
each core then produces its own 48 rows of the final output (for which
V / gate / out-proj are naturally column-sharded).
"""

import math
from contextlib import ExitStack

import numpy as np

import concourse.bacc as bacc
import concourse.bass as bass
import concourse.tile as tile
from concourse import mybir
from concourse.bass_utils import run_bass_kernel_spmd

F32 = mybir.dt.float32
F32R = mybir.dt.float32r
BF16 = mybir.dt.bfloat16
F16 = mybir.dt.float16

D = 128          # pair feature dim (= D_PAIR = D_BIAS)
H = 4            # heads
DH = 32          # head dim
NCORES = 8
L_FULL = 384

# dtype strategy knobs
USE_F32R = True      # reduced-precision fp32 matmuls (full PE rate at N>=256)
QK_BF16 = True       # store Q/K in bf16 (only absolute logit error matters)
NORM_GPSIMD = False   # run the LN normalize pass on GPSIMD (offload DVE)
AV_F16 = True        # fp16 A/V for the attention*V matmul (f32r forbids
                     # col-tiled psum dst offsets; fp16 runs 1cyc/row)
SLAB_F16 = True      # fp16 xhat slabs + projection weights (1cyc/row, FWL)


def _r(ap):
    return ap


def build_program(L, NC, *, has_bq=False, has_bk=False, has_bv=False,
                  has_bo=False, debug=False):
    """Emit the SPMD program (identical for every core)."""
    assert L % 128 == 0
    NIC = L // 128          # number of 128-row chunks of L
    R = L // NC             # columns owned by each core
    nc = bacc.Bacc("TRN2", target_bir_lowering=False, debug=debug,
                   num_devices=NC)

    # ---- kernel I/O (per-core slices, host-prepared) ----
    pc = nc.dram_tensor("pc", [R, L, D], F32, kind="ExternalInput").ap()
    pr = nc.dram_tensor("pr", [R, L, D], F32, kind="ExternalInput").ap()
    bp = nc.dram_tensor("bp", [H, NIC, 128, L], F32, kind="ExternalInput").ap()
    wmat = nc.dram_tensor("wmat", [5 * D, D], F32, kind="ExternalInput").ap()
    wcols = nc.dram_tensor("wcols", [D, 4], F32, kind="ExternalInput").ap()
    bo_r = nc.dram_tensor("bo_r", [1, D], F32, kind="ExternalInput").ap()
    bv3_r = nc.dram_tensor("bv3_r", [1, NIC * D], F32, kind="ExternalInput").ap()
    w16 = nc.dram_tensor("w16", [6, D, D], mybir.dt.float16,
                         kind="ExternalInput").ap()
    out = nc.dram_tensor("out", [R, L, D], F32, kind="ExternalOutput").ap()

    with tile.TileContext(nc) as tc, ExitStack() as ctx:
        consts = ctx.enter_context(tc.tile_pool(name="consts", bufs=1))
        persist = ctx.enter_context(tc.tile_pool(name="persist", bufs=1))
        rot = ctx.enter_context(tc.tile_pool(name="rot", bufs=3))
        dram = ctx.enter_context(tc.tile_pool(name="dram", bufs=1, space="DRAM"))

        # ---- constants ----
        RDT = F32R if USE_F32R else F32
        wqg = consts.tile([128, 5, D], RDT, name="wqg", tag="wqg")
        nc.sync.dma_start(
            out=wqg,
            in_=wmat.rearrange("(a p) d -> p a d", p=128).bitcast(RDT))
        wq_sb = wqg[:, 0, :]
        wk_sb = wqg[:, 1, :]
        wv_sb = wqg[:, 2, :]
        wg_sb = wqg[:, 3, :]
        idr_sb = wqg[:, 4, :]
        w16_sb = consts.tile([128, 6, D], F16, name="w16_sb", tag="w16_sb")
        nc.sync.dma_start(out=w16_sb, in_=w16.rearrange("a p d -> p a d"))
        wo16_sb = w16_sb[:, 0, :]
        id16_sb = w16_sb[:, 1, :]
        if SLAB_F16:
            SDT = F16
            wq_sb = w16_sb[:, 2, :]
            wk_sb = w16_sb[:, 3, :]
            wv_sb = w16_sb[:, 4, :]
            wg_sb = w16_sb[:, 5, :]
            idt_sb = id16_sb
        else:
            SDT = RDT
            idt_sb = idr_sb
        wcols_sb = consts.tile([128, 4], F32, name="wcols_sb", tag="wcols_sb")
        nc.sync.dma_start(out=wcols_sb, in_=wcols)
        bg_col = wcols_sb[:, 0:1]
        eps_col = consts.tile([128, 1], F32, name="eps_col", tag="eps_col")
        nc.vector.memset(eps_col, 1e-5)
        ones_t = consts.tile([1, D], F32, name="ones_t", tag="ones_t")
        if has_bo:
            nc.vector.memset(ones_t, 1.0)
            bo_t = consts.tile([1, D], F32, name="bo_t", tag="bo_t")
            nc.sync.dma_start(out=bo_t, in_=bo_r)
        if has_bv:
            nc.vector.memset(ones_t, 1.0)
            bv3_t = consts.tile([1, NIC * D], F32, name="bv3_t", tag="bv3_t")
            nc.sync.dma_start(out=bv3_t, in_=bv3_r)

        # AllReduce bounce buffers
        arin_t = dram.tile([NIC, H, 128, L], F32, name="arin_t", tag="arin_t")
        arout_t = dram.tile([NIC, H, 128, L], F32, name="arout_t", tag="arout_t",
                            addr_space="Shared" if NC > 4 else "Local")
        arin = [arin_t[ic] for ic in range(NIC)]
        arout = [arout_t[ic] for ic in range(NIC)]
        AVDT = F16 if AV_F16 else F32
        vtd = dram.tile([R, 128, NIC, 128], AVDT, name="vtd", tag="vtd")

        # softmax row-sum buffers
        s_buf = persist.tile([128, H * NIC], F32, name="s_buf", tag="s_buf")
        rcp_buf = persist.tile([128, H * NIC], F32, name="rcp_buf", tag="rcp_buf")

        GS = 8  # slabs per stats group (one sqrt+recip per group)

        def ln_stage1(src_ap, x, g, mvg):
            """DMA row-block x, bn stats -> mvg[:, g, :, :]."""
            xin = rot.tile([128, NIC, D], F32, name=f"xin{x}", tag="xin",
                           bufs=2 * GS + 2)
            nc.sync.dma_start(out=xin,
                              in_=src_ap[x].rearrange("(a p) d -> p a d", p=128))
            for ic in range(NIC):
                st = rot.tile([128, 6], F32, name=f"st{x}_{ic}", tag="st",
                              bufs=4)
                nc.vector.bn_stats(out=st, in_=xin[:, ic, :])
                nc.vector.bn_aggr(out=mvg[:, g, ic, :], in_=st)
            return xin

        def ln_group_rstd(mvg, rsg):
            # rstd = 1/sqrt(var + eps), one op pair for the whole group
            nc.scalar.activation(out=rsg, in_=mvg[:, :, :, 1],
                                 func=mybir.ActivationFunctionType.Sqrt,
                                 bias=eps_col, scale=1.0)
            nc.vector.reciprocal(out=rsg, in_=rsg)

        def ln_stage2(xin, x, g, mvg, rsg, lps_pool, slab, evac_engine):
            xr = rot.tile([128, NIC, D], SDT, name=f"xr{x}", tag="xr", bufs=6)
            for ic in range(NIC):
                nc.vector.tensor_scalar(
                    out=xr[:, ic, :], in0=xin[:, ic, :],
                    scalar1=mvg[:, g, ic, 0:1], scalar2=rsg[:, g, ic:ic + 1],
                    op0=mybir.AluOpType.subtract, op1=mybir.AluOpType.mult)
            lps = lps_pool.tile([128, NIC, 128], SDT, name=f"lps{x}",
                                tag="lps", bufs=2)
            for ic in range(NIC):
                nc.tensor.transpose(out=lps[:, ic, :], in_=xr[:, ic, :],
                                    identity=idt_sb)
            evac_engine(out=slab, in_=lps)

        dve_copy = nc.vector.tensor_copy

        def act_copy(out, in_):
            nc.scalar.copy(out=out, in_=in_)

        qk_dt = BF16 if QK_BF16 else F32

        # =================== pre-AllReduce ===================
        g_pool = ctx.enter_context(tc.tile_pool(name="g_pool", bufs=1))
        with tc.tile_pool(name="ln_ps", bufs=1, space="PSUM") as ln_ps, \
             tc.tile_pool(name="proj_ps", bufs=2, space="PSUM") as proj_ps:  # noqa

            with tc.tile_pool(name="qk_pool", bufs=1) as qk_pool, \
                 tc.tile_pool(name="z_ps", bufs=1, space="PSUM") as z_ps:
                qt, kt = [], []

                def qk_proj(x, slab):
                    qp = proj_ps.tile([128, L], F32, name=f"qp{x}", tag="proj")
                    nc.tensor.matmul(out=qp, lhsT=_r(wq_sb), rhs=_r(slab),
                                     start=True, stop=True)
                    q_sb = qk_pool.tile([128, L], qk_dt, name=f"q{x}",
                                        tag=f"q{x}")
                    if has_bq:
                        nc.scalar.activation(
                            out=q_sb, in_=qp,
                            func=mybir.ActivationFunctionType.Identity,
                            bias=wcols_sb[:, 1:2], scale=1.0)
                    else:
                        nc.scalar.copy(out=q_sb, in_=qp)
                    qt.append(q_sb)
                    kp = proj_ps.tile([128, L], F32, name=f"kp{x}", tag="proj")
                    nc.tensor.matmul(out=kp, lhsT=_r(wk_sb), rhs=_r(slab),
                                     start=True, stop=True)
                    k_sb = qk_pool.tile([128, L], qk_dt, name=f"k{x}",
                                        tag=f"k{x}")
                    if has_bk:
                        nc.scalar.activation(
                            out=k_sb, in_=kp,
                            func=mybir.ActivationFunctionType.Identity,
                            bias=wcols_sb[:, 2:3], scale=1.0)
                    else:
                        nc.scalar.copy(out=k_sb, in_=kp)
                    kt.append(k_sb)

                def vt_proj(x, slab):
                    vp = proj_ps.tile([128, NIC, 128], F32, name=f"vp{x}",
                                      tag="proj")
                    for jc in range(NIC):
                        nc.tensor.matmul(out=vp[:, jc, :],
                                         lhsT=_r(slab[:, jc, :]),
                                         rhs=_r(wv_sb),
                                         start=True, stop=(not has_bv))
                        if has_bv:
                            nc.tensor.matmul(
                                out=vp[:, jc, :], lhsT=ones_t,
                                rhs=bv3_t[:, jc * D:(jc + 1) * D],
                                start=False, stop=True)
                    v_sb = rot.tile([128, NIC, 128], AVDT, name=f"v{x}",
                                    tag="vsp", bufs=4)
                    nc.scalar.copy(out=v_sb, in_=vp)
                    nc.sync.dma_start(out=vtd[x], in_=v_sb)

                for g0 in range(0, R, GS):
                    gn = min(GS, R - g0)
                    mvg = rot.tile([128, GS, NIC, 2], F32, name=f"mvg{g0}",
                                   tag="mvg", bufs=2)
                    rsg = rot.tile([128, GS, NIC], F32, name=f"rsg{g0}",
                                   tag="rsg", bufs=2)
                    xins = [ln_stage1(pc, g0 + g, g, mvg) for g in range(gn)]
                    ln_group_rstd(mvg, rsg)
                    for g in range(gn):
                        x = g0 + g
                        slab = rot.tile([128, NIC, 128], SDT, name=f"xh{x}",
                                        tag="xh", bufs=6)
                        ln_stage2(xins[g], x, g, mvg, rsg, ln_ps, slab,
                                  act_copy)
                        qk_proj(x, slab)
                        vt_proj(x, slab)

                # partial logits Z[h][ic] = sum_x Q_x^T K_x (K=32, row-tiled)
                for ic in range(NIC):
                    zts = [z_ps.tile([128, L], F32, name=f"z{ic}_{h}",
                                     tag=f"z{h}") for h in range(H)]
                    for x in range(R):
                        for h in range(H):
                            nc.tensor.matmul(
                                out=zts[h],
                                lhsT=qt[x][32 * h:32 * h + 32,
                                           ic * 128:(ic + 1) * 128],
                                rhs=kt[x][32 * h:32 * h + 32, :],
                                start=(x == 0), stop=(x == R - 1),
                                tile_position=(32 * h, 0))
                    for h in range(H):
                        bpt = rot.tile([128, L], F32, name=f"bpt{ic}_{h}",
                                       tag="bpt", bufs=3)
                        nc.sync.dma_start(out=bpt, in_=bp[h, ic])
                        zst = rot.tile([128, L], F32, name=f"zst{ic}_{h}",
                                       tag="zst", bufs=3)
                        nc.vector.tensor_add(out=zst, in0=zts[h], in1=bpt)
                        nc.sync.dma_start(out=arin[ic][h], in_=zst)

                # AllReduce the logits (V^T + G production below overlap it)
                nc.gpsimd.collective_compute(
                    "AllReduce", mybir.AluOpType.add,
                    replica_groups=[list(range(NC))],
                    ins=[arin_t.opt()], outs=[arout_t.opt()])

            # gate G[a1]: [hd, a0] = sigmoid(Wg.T @ xhat_row^T + bg)
            gt = []
            for g0 in range(0, R, GS):
                gn = min(GS, R - g0)
                mvg = rot.tile([128, GS, NIC, 2], F32, name=f"mvgr{g0}",
                               tag="mvg", bufs=2)
                rsg = rot.tile([128, GS, NIC], F32, name=f"rsgr{g0}",
                               tag="rsg", bufs=2)
                xins = [ln_stage1(pr, g0 + g, g, mvg) for g in range(gn)]
                ln_group_rstd(mvg, rsg)
                for g in range(gn):
                    x = g0 + g
                    rslab = rot.tile([128, NIC, 128], SDT, name=f"rh{x}",
                                     tag="rh", bufs=4)
                    ln_stage2(xins[g], x, g, mvg, rsg, ln_ps, rslab, dve_copy)
                    gp = proj_ps.tile([128, L], F32, name=f"gp{x}", tag="proj")
                    nc.tensor.matmul(out=gp, lhsT=_r(wg_sb), rhs=_r(rslab),
                                     start=True, stop=True)
                    g_sb = g_pool.tile([128, L], F16, name=f"g{x}",
                                       tag=f"g{x}")
                    nc.scalar.activation(
                        out=g_sb, in_=gp,
                        func=mybir.ActivationFunctionType.Sigmoid,
                        bias=bg_col, scale=1.0)
                    gt.append(g_sb)

        # =================== post-AllReduce ===================
        with tc.tile_pool(name="at_ps", bufs=1, space="PSUM") as at_ps, \
             tc.tile_pool(name="o_ps", bufs=2, space="PSUM") as o_ps, \
             tc.tile_pool(name="u_ps", bufs=2, space="PSUM") as u_ps, \
             tc.tile_pool(name="at_pool", bufs=1) as at_pool:

            # softmax over j (in [i, j] layout) + transpose A -> [j, i]
            at_sb = [[None] * NIC for _ in range(H)]
            for h in range(H):
                atps = [at_ps.tile([128, NIC, 128], F16,
                                   name=f"atp{h}_{jc}", tag=f"at{jc}")
                        for jc in range(NIC)]
                for ic in range(NIC):
                    idx = h * NIC + ic
                    zsum = rot.tile([128, L], F32, name=f"zs{h}_{ic}",
                                    tag="zsum", bufs=3)
                    nc.gpsimd.dma_start(out=zsum, in_=arout[ic][h])
                    e_t = rot.tile([128, L], F16, name=f"e{h}_{ic}",
                                   tag="e", bufs=3)
                    nc.scalar.activation(
                        out=e_t, in_=zsum,
                        func=mybir.ActivationFunctionType.Exp,
                        accum_out=s_buf[:, idx:idx + 1])
                    nc.vector.reciprocal(out=rcp_buf[:, idx:idx + 1],
                                         in_=s_buf[:, idx:idx + 1])
                    nc.vector.tensor_scalar_mul(
                        out=e_t, in0=e_t,
                        scalar1=rcp_buf[:, idx:idx + 1])
                    for jc in range(NIC):
                        nc.tensor.transpose(
                            out=atps[jc][:, ic, :],
                            in_=e_t[:, jc * 128:(jc + 1) * 128],
                            identity=id16_sb)
                for jc in range(NIC):
                    a_sb = at_pool.tile([128, NIC, 128], AVDT,
                                        name=f"at{h}_{jc}",
                                        tag=f"at{h}_{jc}")
                    nc.vector.tensor_copy(out=a_sb, in_=atps[jc])
                    at_sb[h][jc] = a_sb

            # AV (col-tiled over heads) + gate + out-proj + store
            for x in range(R):
                vt_x = rot.tile([128, NIC, 128], AVDT, name=f"vl{x}",
                                tag="vld", bufs=4)
                nc.gpsimd.dma_start(out=vt_x, in_=vtd[x])
                ops_ = o_ps.tile([128, L], F32, name=f"o{x}", tag="o")
                for h in range(H):
                    for jc in range(NIC):
                        nc.tensor.matmul(
                            out=ops_[32 * h:32 * h + 32, :],
                            lhsT=_r(vt_x[:, jc, 32 * h:32 * h + 32]),
                            rhs=_r(at_sb[h][jc]),
                            start=(jc == 0), stop=(jc == NIC - 1),
                            tile_position=(0, 32 * h))
                go = rot.tile([128, L], F16, name=f"go{x}", tag="go",
                              bufs=3)
                nc.vector.tensor_mul(out=go, in0=ops_, in1=gt[x])
                ups = u_ps.tile([128, NIC, 128], F32, name=f"u{x}",
                                tag="u")
                for ic in range(NIC):
                    nc.tensor.matmul(
                        out=ups[:, ic, :],
                        lhsT=go[:, ic * 128:(ic + 1) * 128],
                        rhs=wo16_sb, start=True, stop=(not has_bo))
                    if has_bo:
                        nc.tensor.matmul(out=ups[:, ic, :],
                                         lhsT=ones_t, rhs=bo_t,
                                         start=False, stop=True)
                ut = rot.tile([128, NIC, 128], F32, name=f"ut{x}",
                              tag="ut", bufs=3)
                nc.scalar.copy(out=ut, in_=ups)
                nc.gpsimd.dma_start(
                    out=out[x].rearrange("(a p) d -> p a d", p=128),
                    in_=ut)

    nc.compile()
    return nc


def prep_inputs(pair, bias, ln_g, ln_b, Wq, Wk, Wv, Wb, Wg, bg, Wo, bo,
                L, NC):
    f32 = np.float32
    p2 = np.asarray(pair, f32)[0]
    R = L // NC
    NIC = L // 128
    ln_g = np.asarray(ln_g, f32)
    ln_b = np.asarray(ln_b, f32)
    Wq = np.asarray(Wq, f32)
    Wk = np.asarray(Wk, f32)
    Wv = np.asarray(Wv, f32)
    Wg = np.asarray(Wg, f32)
    Wo = np.asarray(Wo, f32)
    sc_q = 1.0 / math.sqrt(DH)
    sc_k = 1.0 / math.sqrt(L)
    Wq_eff = ln_g[:, None] * Wq * sc_q
    Wk_eff = ln_g[:, None] * Wk * sc_k
    Wv_eff = ln_g[:, None] * Wv
    Wg_eff = ln_g[:, None] * Wg
    bq = (ln_b @ Wq) * sc_q
    bk = (ln_b @ Wk) * sc_k
    bv = ln_b @ Wv
    bgE = ln_b @ Wg + np.asarray(bg, f32)
    bo = np.asarray(bo, f32)
    BP = np.einsum("ijk,kh->hij", np.asarray(bias, f32)[0],
                   np.asarray(Wb, f32)).astype(f32)
    wmat = np.concatenate(
        [Wq_eff, Wk_eff, Wv_eff, Wg_eff, np.eye(D, dtype=f32)], 0
    ).astype(f32)
    wcols = np.stack([bgE, bq, bk, bv], 1).astype(f32)
    pcT = np.ascontiguousarray(p2.transpose(1, 0, 2))
    flags = dict(has_bq=bool(np.any(bq != 0)), has_bk=bool(np.any(bk != 0)),
                 has_bv=bool(np.any(bv != 0)), has_bo=bool(np.any(bo != 0)))
    in_maps = []
    for c in range(NC):
        sl = slice(c * R, (c + 1) * R)
        bp_c = np.zeros((H, L, L), f32)
        bp_c[:, sl, :] = BP[:, sl, :]
        in_maps.append({
            "pc": np.ascontiguousarray(pcT[sl]),
            "pr": np.ascontiguousarray(p2[sl]),
            "bp": np.ascontiguousarray(bp_c.reshape(H, NIC, 128, L)),
            "wmat": wmat,
            "wcols": wcols,
            "bo_r": bo.reshape(1, D).astype(f32),
            "bv3_r": np.tile(bv, NIC).reshape(1, NIC * D).astype(f32),
            "w16": np.stack([Wo, np.eye(D, dtype=f32), Wq_eff, Wk_eff,
                             Wv_eff, Wg_eff], 0).astype(np.float16),
        })
    return in_maps, flags


def gather_output(results, L, NC):
    full = np.concatenate([r["out"] for r in results], axis=0)
    return full.reshape(1, L, L, D)


_CACHED = {}
TRACE = False          # set True (e.g. from test.py) to capture an NTFF trace
LAST_RESULT = None     # BassKernelResults of the most recent kernel() call


def kernel(**inputs):
    global LAST_RESULT
    L = int(np.asarray(inputs["pair"]).shape[1])
    NC = NCORES
    in_maps, flags = prep_inputs(
        inputs["pair"], inputs["bias"], inputs["ln_g"], inputs["ln_b"],
        inputs["Wq"], inputs["Wk"], inputs["Wv"], inputs["Wb"], inputs["Wg"],
        inputs["bg"], inputs["Wo"], inputs["bo"], L, NC)
    key = (L, NC, tuple(sorted(flags.items())))
    if key not in _CACHED:
        _CACHED[key] = build_program(L, NC, **flags)
    nc = _CACHED[key]
    res = run_bass_kernel_spmd(nc, in_maps, core_ids=list(range(NC)),
                               trace=TRACE)
    LAST_RESULT = res
    return gather_output(res.results, L, NC)

